# revision 23
# baseline (speedup 1.0000x reference)
"""MACE GNN layer on 8 Trainium2 NeuronCores (Bass/Tile).

Strategy
--------
- Nodes sharded by receiver-core (node n -> core n // (N/8)); within each core
  nodes are re-ordered grouped by species (fixed-size padded segments) so the
  per-species selector / symmetric-contraction become segment matmuls.
- Edges sharded by receiver core and sorted into 128-node receiver windows so
  the scatter-add becomes PSUM-accumulated one-hot matmuls on the PE.
- The Y1 (unit edge vector) mixing of the tensor-product messages is folded
  into *weighted* one-hot scatter matrices (coefficients 1, Y1_x, Y1_y, Y1_z),
  so the DVE only computes the 11 elementwise radial-weight products.
- The up-projected node feature table (bf16, [ncore*NLP, 256] rows
  [s | v_x | v_y | v_z]) is built per-core and AllGathered through HBM; sender
  features are fetched with SWDGE dma_gather (512B rows).
"""

import math
import numpy as np
import ml_dtypes

import concourse.bacc as bacc
import concourse.bass as bass
import concourse.mybir as mybir
from concourse import tile
from concourse.tile_rust import add_dep_helper
from concourse.bass_utils import run_bass_kernel_spmd

F32 = mybir.dt.float32
BF16 = mybir.dt.bfloat16
I16 = mybir.dt.int16
BFNP = ml_dtypes.bfloat16

CORES = 8
C = 64          # channels
S = 10          # species
RB = 8          # radial basis
RH = 64         # radial hidden
NPATH = 5
AVG_NEIGH = 16.0
G = 8           # chunks (128 edges each) per supertile
AF = mybir.ActivationFunctionType
OP = mybir.AluOpType


# --------------------------------------------------------------------------
# host-side preparation
# --------------------------------------------------------------------------

def _prep(vectors, node_s, node_v, radial_embedding, node_specie, senders,
          receivers):
    N = node_s.shape[0]
    E = vectors.shape[0]
    assert N % CORES == 0
    NLOC = N // CORES

    vectors = np.asarray(vectors, np.float32)
    node_s = np.asarray(node_s, np.float32)
    node_v = np.asarray(node_v, np.float32)
    radial_embedding = np.asarray(radial_embedding, np.float32)
    specie = np.asarray(node_specie).astype(np.int64)
    senders = np.asarray(senders).astype(np.int64)
    receivers = np.asarray(receivers).astype(np.int64)

    core_of = np.arange(N) // NLOC
    cnt = np.zeros((CORES, S), np.int64)
    for k in range(CORES):
        cnt[k] = np.bincount(specie[k * NLOC:(k + 1) * NLOC], minlength=S)
    SEGP = int(math.ceil(cnt.max() / 64.0)) * 64
    NLP = S * SEGP                      # padded local node count
    assert NLP % 128 == 0
    NWIN = NLP // 128
    assert CORES * NLP < 32768, "table rows must fit int16"

    loc_pos = np.zeros(N, np.int64)
    for k in range(CORES):
        sl = slice(k * NLOC, (k + 1) * NLOC)
        sp = specie[sl]
        order = np.argsort(sp, kind="stable")
        ranks = np.empty(NLOC, np.int64)
        seg_off = np.zeros(S, np.int64)
        seg_off[1:] = np.cumsum(cnt[k])[:-1]
        ranks[order] = np.arange(NLOC) - seg_off[sp[order]]
        loc_pos[sl] = sp * SEGP + ranks
    grow = core_of * NLP + loc_pos      # global table row

    # ---- edge bucketing by (receiver core, 128-node window) ----
    ek = receivers // NLOC
    loc_r = loc_pos[receivers]
    win_e = loc_r // 128
    col_e = loc_r % 128

    # half-window (64-node) chunk accounting; chunks interleave halves so the
    # PE can overlap col-tiled scatter matmuls of the two halves.
    half_e = (loc_r % 128) // 64
    hw_e = win_e * 2 + half_e
    cnt_kh = np.zeros((CORES, NWIN * 2), np.int64)
    for k in range(CORES):
        cnt_kh[k] = np.bincount(hw_e[ek == k], minlength=NWIN * 2)
    M_wh = np.ceil(cnt_kh.max(axis=0) / 128.0).astype(np.int64).reshape(NWIN, 2)
    M_w = M_wh.sum(axis=1)
    CT = int(M_w.sum())
    pad_ch = (G - CT % G) % G
    if pad_ch:
        nz = np.nonzero(M_w)[0]
        wl = nz[-1] if len(nz) else NWIN - 1
        M_wh[wl, 0] += pad_ch
        M_w[wl] += pad_ch
        CT += pad_ch
    NST = CT // G
    EPAD = CT * 128

    chunk_win = np.repeat(np.arange(NWIN), M_w)      # [CT]
    chunk_half = np.zeros(CT, np.int64)
    half_first = np.zeros(CT, bool)
    half_last = np.zeros(CT, bool)
    win_first = np.zeros(CT, bool)
    win_last = np.zeros(CT, bool)
    # global chunk index for (w, h, i-th chunk of that half)
    hw_chunk_base = {}
    off = 0
    for w in range(NWIN):
        m0, m1 = int(M_wh[w, 0]), int(M_wh[w, 1])
        order = []
        for i in range(max(m0, m1)):
            if i < m0:
                order.append(0)
            if i < m1:
                order.append(1)
        seen = {0: 0, 1: 0}
        for i, h in enumerate(order):
            cc = off + i
            chunk_half[cc] = h
            if seen[h] == 0:
                half_first[cc] = True
            if seen[h] == ([m0, m1][h] - 1):
                half_last[cc] = True
            hw_chunk_base[(w, h, seen[h])] = cc
            seen[h] += 1
        if order:
            win_first[off] = True
            win_last[off + len(order) - 1] = True
        off += len(order)

    # map (w, h, chunk#) -> global chunk start slot
    chunk_base_arr = np.zeros((NWIN, 2, int(M_wh.max()) + 1), np.int64)
    for (w, h, i), cc in hw_chunk_base.items():
        chunk_base_arr[w, h, i] = cc
    per_core = []
    for k in range(CORES):
        idx = np.nonzero(ek == k)[0]
        hw = hw_e[idx]
        order = np.argsort(hw, kind="stable")
        idx = idx[order]
        hw = hw[order]
        hc = np.bincount(hw, minlength=NWIN * 2)
        starts = np.concatenate([[0], np.cumsum(hc)[:-1]])
        rank = np.arange(len(idx)) - np.repeat(starts, hc)
        wv, hv = hw // 2, hw % 2
        slot = chunk_base_arr[wv, hv, rank // 128] * 128 + rank % 128
        slot_edge = np.full(EPAD, -1, np.int64)
        slot_edge[slot] = idx

        mask = slot_edge >= 0
        se = np.where(mask, slot_edge, 0)
        vec_f = np.where(mask[:, None], vectors[se], 0.0).astype(np.float32)
        rad_f = np.where(mask[:, None], radial_embedding[se], 0.0).astype(np.float32)
        oh_f = np.where(mask, (col_e[se] % 64).astype(np.float32), -1.0).astype(np.float32)
        gx_f = np.where(mask, grow[senders[se]], 0).astype(np.int16)

        vec_st = vec_f.reshape(NST, G, 128, 3).transpose(0, 2, 1, 3).reshape(
            NST, 128, G * 3).copy()
        rad_st = rad_f.reshape(NST, G * 128, RB).transpose(0, 2, 1).copy()
        oh_st = oh_f.reshape(NST, G, 128).transpose(0, 2, 1).astype(BFNP).copy()
        gx_st = np.tile(gx_f.reshape(NST, 64, 16).transpose(0, 2, 1),
                        (1, 8, 1)).copy()
        # pack [oh(bf16) | gx(i16)] into one f32-typed DMA payload
        PK = G // 2 + 32
        epack = np.zeros((NST, 128, PK), np.float32)
        pk16 = epack.view(np.uint16).reshape(NST, 128, PK * 2)
        pk16[:, :, 0:G] = oh_st.view(np.uint16)
        pk16[:, :, G:G + 64] = gx_st.view(np.uint16)
        vecs_all = vec_st.transpose(1, 0, 2).reshape(128, NST * G * 3).copy()

        nsT = np.zeros((C, NLP), np.float32)
        nvT = np.zeros((C, 3 * NLP), np.float32)
        sl = slice(k * NLOC, (k + 1) * NLOC)
        lp = loc_pos[sl]
        nsT[:, lp] = node_s[sl].T
        for d in range(3):
            nvT[:, d * NLP + lp] = node_v[sl, :, d].T
        per_core.append(dict(epack=epack, rad=rad_st, vca=vecs_all, nsT=nsT, nvT=nvT))

    meta = dict(N=N, E=E, NLOC=NLOC, SEGP=SEGP, NLP=NLP, NWIN=NWIN,
                NST=NST, CT=CT,
                chunk_win=chunk_win.tolist(), win_first=win_first.tolist(),
                win_last=win_last.tolist(), loc_pos=loc_pos,
                chunk_half=chunk_half.tolist(),
                half_first=half_first.tolist(), half_last=half_last.tolist())
    return per_core, meta


def _prep_weights(W_up_s, W_up_v, Wr1, br1, Wr2, br2, W_down_s, W_down_v,
                  Wsel_s, Wsel_v, Wsym_s, Wsym_v, Wpost_s, Wpost_v, W_read):
    f32 = lambda x: np.asarray(x, np.float32)
    w = {}
    w["wups"] = f32(W_up_s).copy()
    w["wupv"] = f32(W_up_v).copy()
    w["wr1"] = f32(Wr1).copy()
    w["br1"] = f32(br1).reshape(RH, 1).copy()
    w["wr2a"] = np.concatenate([f32(Wr2), f32(br2)[None, :]], 0).astype(BFNP)
    w["wdns"] = (f32(W_down_s) / AVG_NEIGH).copy()
    w["wdnv"] = (f32(W_down_v) / AVG_NEIGH).copy()
    w["wsels"] = f32(Wsel_s).transpose(1, 0, 2).reshape(C, S * C).copy()
    w["wselv"] = f32(Wsel_v).transpose(1, 0, 2).reshape(C, S * C).copy()
    w["wsyms"] = f32(Wsym_s).transpose(2, 0, 1).reshape(C, S * NPATH).copy()
    w["wsymv"] = f32(Wsym_v).transpose(2, 0, 1).reshape(C, S * 4).copy()
    w["wposts"] = f32(Wpost_s).copy()
    w["wpostv"] = f32(Wpost_v).copy()
    w["wread"] = f32(W_read).copy()
    w["iota"] = np.broadcast_to(np.arange(64, dtype=np.float32),
                                (128, 64)).astype(BFNP).reshape(128, 1, 64).copy()
    w["eye"] = np.eye(128, dtype=np.float32)
    return w


WDTYPES = dict(wups=F32, wupv=F32, wr1=F32, br1=F32, wr2a=BF16, wdns=F32,
               wdnv=F32, wsels=F32, wselv=F32, wsyms=F32, wsymv=F32,
               wposts=F32, wpostv=F32, wread=F32, iota=BF16, eye=F32)
WSHAPES = dict(wups=[C, C], wupv=[C, C], wr1=[RB, RH], br1=[RH, 1],
               wr2a=[RH + 1, NPATH * C], wdns=[C, C], wdnv=[C, C],
               wsels=[C, S * C], wselv=[C, S * C], wsyms=[C, S * NPATH],
               wsymv=[C, S * 4], wposts=[C, C], wpostv=[C, C], wread=[C, 1],
               iota=[128, 1, 64], eye=[128, 128])


# --------------------------------------------------------------------------
# device program
# --------------------------------------------------------------------------

def _build(meta, single_core=False, skip=(), use_silu=True):
    NLP, NWIN, NST = meta["NLP"], meta["NWIN"], meta["NST"]
    SEGP = meta["SEGP"]
    chunk_win = meta["chunk_win"]
    win_first, win_last = meta["win_first"], meta["win_last"]
    chunk_half = meta["chunk_half"]
    half_first, half_last = meta["half_first"], meta["half_last"]
    TROWS = CORES * NLP

    nc = bacc.Bacc("TRN2", target_bir_lowering=False, debug=False,
                   num_devices=1 if single_core else CORES)

    din = {}
    PK = G // 2 + 32
    din["epack"] = nc.dram_tensor("epack", [NST, 128, PK], F32, kind="ExternalInput")
    din["rad"] = nc.dram_tensor("rad", [NST, RB, G * 128], F32, kind="ExternalInput")
    din["vca"] = nc.dram_tensor("vca", [128, NST * G * 3], F32, kind="ExternalInput")
    din["nsT"] = nc.dram_tensor("nsT", [C, NLP], F32, kind="ExternalInput")
    din["nvT"] = nc.dram_tensor("nvT", [C, 3 * NLP], F32, kind="ExternalInput")
    for n, sh in WSHAPES.items():
        din[n] = nc.dram_tensor(n, sh, WDTYPES[n], kind="ExternalInput")
    out_s = nc.dram_tensor("out_s", [C, NLP], F32, kind="ExternalOutput")
    out_v = nc.dram_tensor("out_v", [C, 3 * NLP], F32, kind="ExternalOutput")
    out_r = nc.dram_tensor("out_r", [1, NLP], F32, kind="ExternalOutput")

    with tile.TileContext(nc) as tc:
        with (
            tc.tile_pool(name="cw", bufs=1) as cw,
            tc.tile_pool(name="dram", bufs=1, space="DRAM") as dram,
            tc.tile_pool(name="agg", bufs=1) as agp,
            tc.tile_pool(name="ppacc", bufs=4, space="PSUM") as ppacc,
            tc.tile_pool(name="ppmm", bufs=2, space="PSUM") as ppmm,
            tc.tile_pool(name="ppw", bufs=2, space="PSUM") as ppw,
        ):
            W = {}
            for n, sh in WSHAPES.items():
                W[n] = cw.tile(sh, WDTYPES[n], tag=n, name=n)
                nc.sync.dma_start(W[n][:], din[n][:])

            tbl_slice = dram.tile([NLP, 256], BF16, tag="tsl")
            tbl_full = dram.tile([TROWS, 256], BF16, tag="tfl",
                                 addr_space="Shared")
            aggT = {p: agp.tile([C, NLP], F32, tag=f"aggT{p}", name=f"aggT{p}") for p in range(4)}
            for p in range(4):
                nc.vector.memset(aggT[p][:], 0.0)

            # ---------------- phase A: node table + AllGather ------------
            with tc.tile_pool(name="pa", bufs=1) as pa, \
                 tc.tile_pool(name="pat", bufs=2) as pat:
                nsT = pa.tile([C, NLP], F32, tag="nsT")
                nvT = pa.tile([C, 3 * NLP], F32, tag="nvT")
                nc.sync.dma_start(nsT[:], din["nsT"][:])
                nc.sync.dma_start(nvT[:], din["nvT"][:])
                for w in range(NWIN):
                    pu = ppacc.tile([128, 256], F32, tag="acc")
                    cols = slice(w * 128, (w + 1) * 128)
                    nc.tensor.matmul(out=pu[:, 0:C], lhsT=nsT[:, cols],
                                     rhs=W["wups"][:], start=True, stop=True)
                    for d in range(3):
                        nc.tensor.matmul(
                            out=pu[:, C * (1 + d):C * (2 + d)],
                            lhsT=nvT[:, d * NLP + w * 128:d * NLP + (w + 1) * 128],
                            rhs=W["wupv"][:], start=True, stop=True)
                    tb = pat.tile([128, 256], BF16, tag="tb")
                    nc.scalar.copy(tb[:], pu[:])
                    nc.sync.dma_start(tbl_slice[w * 128:(w + 1) * 128, :], tb[:])
                if single_core:
                    nc.sync.dma_start(tbl_full[0:NLP, :], tbl_slice[:])
                else:
                    nc.gpsimd.collective_compute(
                        "AllGather", OP.bypass,
                        replica_groups=[list(range(CORES))],
                        ins=[tbl_slice.opt()], outs=[tbl_full.opt()])

            # ---------------- phase B: edges ------------------------------
            agg_ps = {}
            with tc.tile_pool(name="ed", bufs=3) as ed, \
                 tc.tile_pool(name="yb", bufs=1) as yb, \
                 tc.tile_pool(name="hga", bufs=1) as hga:
                hsl = []
                for hi in range(3):
                    hsx = hga.tile([RH + 1, G * 128], BF16, tag=f"h{hi}",
                                   name=f"hs{hi}")
                    nc.vector.memset(hsx[RH:RH + 1, :], 1.0)
                    hsl.append(hsx)
                # ---- B0: all Y1 normalizations up-front (keeps Sqrt out of
                # the edge loop so the act-table stays on the silu/copy set)
                vca = yb.tile([128, NST * G * 3], F32, tag="vca")
                y1a = yb.tile([128, NST, G, 3], BF16, tag="y1a")
                nc.sync.dma_start(vca[:], din["vca"][:])
                b0_sqrts = []
                for st in range(NST):
                    vv_ = vca[:, st * G * 3:(st + 1) * G * 3].rearrange(
                        "p (g d) -> p g d", d=3)
                    sq = ed.tile([128, G, 3], F32, tag="sq")
                    nrm = ed.tile([128, G], F32, tag="nrm")
                    rno = ed.tile([128, G], F32, tag="rno")
                    nc.vector.tensor_mul(sq[:], vv_, vv_)
                    nc.vector.tensor_add(nrm[:], sq[:, :, 0], sq[:, :, 1])
                    nc.vector.tensor_add(nrm[:], nrm[:], sq[:, :, 2])
                    b0_sqrts.append(nc.scalar.sqrt(nrm[:], nrm[:]).ins)
                    nc.vector.tensor_scalar_add(nrm[:], nrm[:], 1e-9)
                    nc.vector.reciprocal(rno[:], nrm[:])
                    nc.vector.tensor_mul(y1a[:, st], vv_,
                                         rno[:].to_broadcast([128, G, 3]))
                PK = G // 2 + 32
                for st in range(NST):
                    ein = ed.tile([128, PK], F32, tag="ein")
                    rad = ed.tile([RB, G * 128], F32, tag="rad")
                    nc.sync.dma_start(ein[:], din["epack"][st])
                    nc.sync.dma_start(rad[:], din["rad"][st])
                    oh = ein[:, 0:G // 2].bitcast(BF16)
                    gx = ein[:, G // 2:PK].bitcast(I16)
                    y1b = y1a[:, st]

                    feat = ed.tile([128, G, 256], BF16, tag="feat")
                    if "gather" not in skip:
                      nc.gpsimd.dma_gather(out_ap=feat[:], in_ap=tbl_full[:],
                                         idxs_ap=gx, num_idxs=G * 128,
                                         num_idxs_reg=G * 128, elem_size=256)


                    # one-hot scatter matrices (bf16): O, O*Y1_d (64 cols)
                    Og = ed.tile([128, G, 256], BF16, tag="Og")
                    if "ob" not in skip:
                        nc.vector.tensor_tensor(
                            Og[:, :, 0:64],
                            oh.to_broadcast([128, G, 64]),
                            W["iota"][:].to_broadcast([128, G, 64]),
                            op=OP.is_equal)
                        for d in range(3):
                            nc.vector.tensor_tensor(
                                Og[:, :, 64 * (d + 1):64 * (d + 2)],
                                Og[:, :, 0:64],
                                y1b[:, :, d].to_broadcast([128, G, 64]),
                                op=OP.mult)

                    # radial MLP layer 1 (h^T) + silu into h_aug
                    haug = hsl[st % 3]
                    sgt = None if use_silu else ed.tile([RH, G * 128], BF16, tag="sgt")
                    for hh in range(0 if "radial" in skip else G * 128 // 512):
                        hs = slice(hh * 512, (hh + 1) * 512)
                        ph = ppmm.tile([RH, 512], F32, tag="mm")
                        nc.tensor.matmul(out=ph[:], lhsT=W["wr1"][:],
                                         rhs=rad[:, hs], start=True, stop=True)
                        if use_silu:
                            si = nc.scalar.activation(haug[0:RH, hs], ph[:],
                                                      AF.Silu,
                                                      bias=W["br1"][:, 0:1],
                                                      scale=1.0)
                            if st == 0 and hh == 0:
                                # keep all B0 Sqrt ops ahead of Silu on ACT so
                                # the act-func table loads exactly twice
                                for a, b in zip(b0_sqrts, b0_sqrts[1:]):
                                    add_dep_helper(b, a, sync=False,
                                                   reason="b0 sqrt order")
                                add_dep_helper(si.ins, b0_sqrts[-1], sync=False,
                                               reason="silu after b0 sqrts")
                        else:
                            nc.scalar.activation(sgt[:, hs], ph[:], AF.Sigmoid,
                                                 bias=W["br1"][:, 0:1], scale=1.0)
                            nc.scalar.activation(haug[0:RH, hs], ph[:], AF.Identity,
                                                 bias=W["br1"][:, 0:1], scale=1.0)
                    if not use_silu:
                        nc.vector.tensor_mul(haug[0:RH, :], haug[0:RH, :], sgt[:])

                    # radial layer 2 per chunk + cast to bf16
                    wbf = ed.tile([128, G, NPATH * C], BF16, tag="wbf")
                    for j in range(0 if "radial" in skip else G):
                        pw = ppw.tile([128, NPATH * C], F32, tag="w")
                        nc.tensor.matmul(out=pw[:],
                                         lhsT=haug[:, j * 128:(j + 1) * 128],
                                         rhs=W["wr2a"][:], start=True, stop=True)
                        nc.scalar.copy(wbf[:, j, :], pw[:])

                    # payload planes (16 slots of 64 cols)
                    upl = ed.tile([128, G, 1024], BF16, tag="upl")
                    ss = feat[:, :, 0:C]
                    sv = [feat[:, :, C * (1 + d):C * (2 + d)] for d in range(3)]
                    wp = [wbf[:, :, C * p:C * (p + 1)] for p in range(NPATH)]
                    slot = lambda i: upl[:, :, C * i:C * (i + 1)]
                    nopmul = (lambda *a, **k: None)
                    mul = nopmul if "muls" in skip else nc.vector.tensor_mul
                    mul(slot(0), wp[0], ss)            # u0
                    mul(slot(1), wp[2], sv[0])         # u2_0
                    mul(slot(2), wp[2], sv[1])         # u2_1
                    mul(slot(3), wp[2], sv[2])         # u2_2
                    mul(slot(4), wp[3], sv[0])         # u3_0
                    mul(slot(5), wp[1], ss)            # u1
                    mul(slot(6), wp[4], sv[2])         # u4_2
                    mul(slot(13), wp[4], sv[1])        # u4_1
                    mul(slot(8), wp[3], sv[1])         # u3_1
                    mul(slot(11), wp[4], sv[0])        # u4_0
                    mul(slot(12), wp[3], sv[2])        # u3_2
                    if "muls" not in skip:
                        nc.scalar.mul(slot(7), slot(13), -1.0)
                        nc.vector.tensor_scalar_mul(slot(9), slot(6), -1.0)
                        nc.vector.tensor_scalar_mul(slot(14), slot(11), -1.0)
                        nc.vector.tensor_copy(slot(10), slot(5))
                        nc.scalar.copy(slot(15), slot(5))

                    # weighted one-hot scatter into window PSUM (col-tiled
                    # by 64-node half-window)
                    for j in range(G):
                        cc = st * G + j
                        w = chunk_win[cc]
                        h = chunk_half[cc]
                        if half_first[cc]:
                            agg_ps[(w, h)] = ppacc.tile(
                                [64, 256], F32, tag="acc", name=f"agg{w}_{h}")
                        pt = agg_ps[(w, h)]
                        for gs in (() if "scatter" in skip else range(4)):
                            nc.tensor.matmul(
                                out=pt[:],
                                lhsT=Og[:, j, 64 * gs:64 * (gs + 1)],
                                rhs=upl[:, j, 256 * gs:256 * (gs + 1)],
                                start=(half_first[cc] and gs == 0),
                                stop=(half_last[cc] and gs == 3))
                        if win_last[cc]:
                            asb = ed.tile([128, 256], F32, tag="asb")
                            for hh2 in range(2):
                                if (w, hh2) in agg_ps:
                                    nc.scalar.copy(asb[64 * hh2:64 * (hh2 + 1), :],
                                                   agg_ps[(w, hh2)][:])
                                else:
                                    nc.vector.memset(
                                        asb[64 * hh2:64 * (hh2 + 1), :], 0.0)
                            wcols = slice(w * 128, (w + 1) * 128)
                            for half in range(2):
                                ptr = ppw.tile([128, 128], F32, tag="w")
                                nc.tensor.matmul(
                                    out=ptr[:],
                                    lhsT=asb[:, 128 * half:128 * (half + 1)],
                                    rhs=W["eye"][:], start=True, stop=True)
                                nc.scalar.copy(aggT[2 * half][:, wcols],
                                               ptr[0:C, :])
                                nc.scalar.copy(aggT[2 * half + 1][:, wcols],
                                               ptr[C:128, :])

            # ---------------- phase C: node pipeline ----------------------
            with tc.tile_pool(name="nd", bufs=1) as nd, \
                 tc.tile_pool(name="sg", bufs=2) as sg:
                s1 = nd.tile([C, NLP], F32, tag="t1")
                v1 = nd.tile([C, 3 * NLP], F32, tag="t2")
                for t0 in range(0, NLP, 512):
                    tl = slice(t0, min(t0 + 512, NLP))
                    n = tl.stop - tl.start
                    pd = ppmm.tile([C, 512], F32, tag="mm")
                    nc.tensor.matmul(out=pd[:, 0:n], lhsT=W["wdns"][:],
                                     rhs=aggT[0][:, tl], start=True, stop=True)
                    nc.scalar.copy(s1[:, tl], pd[:, 0:n])
                    for d in range(3):
                        pv = ppmm.tile([C, 512], F32, tag="mm")
                        nc.tensor.matmul(out=pv[:, 0:n], lhsT=W["wdnv"][:],
                                         rhs=aggT[1 + d][:, tl],
                                         start=True, stop=True)
                        nc.scalar.copy(v1[:, d * NLP + tl.start:d * NLP + tl.stop],
                                       pv[:, 0:n])

                z = nd.tile([C, NLP], F32, tag="z")
                vq = nd.tile([C, 3 * NLP], F32, tag="vq")
                stt = nc.vector.scalar_tensor_tensor
                for s in range(S):
                    seg = slice(s * SEGP, (s + 1) * SEGP)
                    # selector matmuls for this species segment
                    seg_se = sg.tile([C, SEGP], F32, tag="se")
                    seg_ve = sg.tile([C, 3 * SEGP], F32, tag="ve")
                    ps = ppw.tile([C, SEGP], F32, tag="w")
                    nc.tensor.matmul(out=ps[:],
                                     lhsT=W["wsels"][:, s * C:(s + 1) * C],
                                     rhs=s1[:, seg], start=True, stop=True)
                    nc.scalar.copy(seg_se[:], ps[:])
                    for d in range(3):
                        pv = ppw.tile([C, SEGP], F32, tag="w")
                        nc.tensor.matmul(
                            out=pv[:], lhsT=W["wselv"][:, s * C:(s + 1) * C],
                            rhs=v1[:, d * NLP + seg.start:d * NLP + seg.stop],
                            start=True, stop=True)
                        nc.scalar.copy(seg_ve[:, d * SEGP:(d + 1) * SEGP], pv[:])

                    # symmetric contraction for this segment
                    sed = [seg_ve[:, d * SEGP:(d + 1) * SEGP] for d in range(3)]
                    se2 = sg.tile([C, SEGP], F32, tag="se2")
                    vv = sg.tile([C, SEGP], F32, tag="vv")
                    tA = sg.tile([C, SEGP], F32, tag="tA")
                    tB = sg.tile([C, SEGP], F32, tag="tB")
                    cs = lambda p: W["wsyms"][:, s * NPATH + p:s * NPATH + p + 1]
                    cv = lambda p: W["wsymv"][:, s * 4 + p:s * 4 + p + 1]
                    nc.vector.tensor_mul(se2[:], seg_se[:], seg_se[:])
                    nc.vector.tensor_mul(vv[:], sed[0], sed[0])
                    nc.vector.tensor_mul(tA[:], sed[1], sed[1])
                    nc.vector.tensor_add(vv[:], vv[:], tA[:])
                    nc.vector.tensor_mul(tA[:], sed[2], sed[2])
                    nc.vector.tensor_add(vv[:], vv[:], tA[:])
                    # z = se*(c0 + c1*se + c2*se2) + vv*(c3 + c4*se)
                    stt(tA[:], seg_se[:], cs(1), cs(0).to_broadcast([C, SEGP]),
                        op0=OP.mult, op1=OP.add)
                    stt(tA[:], se2[:], cs(2), tA[:], op0=OP.mult, op1=OP.add)
                    nc.vector.tensor_mul(z[:, seg], seg_se[:], tA[:])
                    stt(tB[:], seg_se[:], cs(4), cs(3).to_broadcast([C, SEGP]),
                        op0=OP.mult, op1=OP.add)
                    nc.vector.tensor_mul(tB[:], vv[:], tB[:])
                    nc.vector.tensor_add(z[:, seg], z[:, seg], tB[:])
                    # q = cv0 + cv1*se + cv2*se2 + cv3*vv ; vq_d = ve_d * q
                    stt(tA[:], seg_se[:], cv(1), cv(0).to_broadcast([C, SEGP]),
                        op0=OP.mult, op1=OP.add)
                    stt(tA[:], se2[:], cv(2), tA[:], op0=OP.mult, op1=OP.add)
                    stt(tA[:], vv[:], cv(3), tA[:], op0=OP.mult, op1=OP.add)
                    for d in range(3):
                        nc.vector.tensor_mul(
                            vq[:, d * NLP + seg.start:d * NLP + seg.stop],
                            sed[d], tA[:])

                # post linear + readout
                oS = nd.tile([C, NLP], F32, tag="t1")
                oV = nd.tile([C, 3 * NLP], F32, tag="t2")
                oR = nd.tile([1, NLP], F32, tag="oR")
                for t0 in range(0, NLP, 512):
                    tl = slice(t0, min(t0 + 512, NLP))
                    n = tl.stop - tl.start
                    pp = ppmm.tile([C, 512], F32, tag="mm")
                    nc.tensor.matmul(out=pp[:, 0:n], lhsT=W["wposts"][:],
                                     rhs=z[:, tl], start=True, stop=True)
                    nc.scalar.copy(oS[:, tl], pp[:, 0:n])
                    for d in range(3):
                        pv = ppmm.tile([C, 512], F32, tag="mm")
                        nc.tensor.matmul(
                            out=pv[:, 0:n], lhsT=W["wpostv"][:],
                            rhs=vq[:, d * NLP + tl.start:d * NLP + tl.stop],
                            start=True, stop=True)
                        nc.scalar.copy(oV[:, d * NLP + tl.start:d * NLP + tl.stop],
                                       pv[:, 0:n])
                    pr = ppw.tile([1, 512], F32, tag="w")
                    nc.tensor.matmul(out=pr[:, 0:n], lhsT=W["wread"][:],
                                     rhs=oS[:, tl], start=True, stop=True)
                    nc.scalar.copy(oR[:, tl], pr[:, 0:n])

                nc.sync.dma_start(out_s[:], oS[:])
                nc.sync.dma_start(out_v[:], oV[:])
                nc.sync.dma_start(out_r[:], oR[:])

    nc.compile()
    return nc


# --------------------------------------------------------------------------
# entry point
# --------------------------------------------------------------------------

def _in_maps(per_core, weights):
    maps = []
    for k in range(CORES):
        mm = dict(per_core[k])
        mm.update(weights)
        maps.append(mm)
    return maps


def _assemble(results, meta):
    N, NLOC, NLP = meta["N"], meta["NLOC"], meta["NLP"]
    loc_pos = meta["loc_pos"]
    s_full = np.zeros((N, C), np.float32)
    v_full = np.zeros((N, C, 3), np.float32)
    r_full = np.zeros((N, 1, 1), np.float32)
    for k in range(CORES):
        sl = slice(k * NLOC, (k + 1) * NLOC)
        lp = loc_pos[sl]
        s_full[sl] = results[k]["out_s"][:, lp].T
        ov = results[k]["out_v"].reshape(C, 3, NLP)
        v_full[sl] = ov[:, :, lp].transpose(2, 0, 1)
        r_full[sl, 0, 0] = results[k]["out_r"][0, lp]
    return r_full, s_full, v_full


def prepare(inputs):
    per_core, meta = _prep(
        inputs["vectors"], inputs["node_s"], inputs["node_v"],
        inputs["radial_embedding"], inputs["node_specie"],
        inputs["senders"], inputs["receivers"])
    weights = _prep_weights(
        inputs["W_up_s"], inputs["W_up_v"], inputs["Wr1"], inputs["br1"],
        inputs["Wr2"], inputs["br2"], inputs["W_down_s"], inputs["W_down_v"],
        inputs["Wsel_s"], inputs["Wsel_v"], inputs["Wsym_s"], inputs["Wsym_v"],
        inputs["Wpost_s"], inputs["Wpost_v"], inputs["W_read"])
    return _in_maps(per_core, weights), meta


def kernel(**inputs):
    maps, meta = prepare(inputs)
    nc = _build(meta)
    res = run_bass_kernel_spmd(nc, maps, core_ids=list(range(CORES)))
    return _assemble(res.results, meta)
